# revision 13
# baseline (speedup 1.0000x reference)
"""Trainium2 Bass kernel for nn_MinkowskiSpMiddleResNetFHD (sparse 3D ResNet).

Strategy (v0):
- Host (numpy): build voxel hash tables / kernel maps per level, shard output
  rows across 8 cores, localize gather indices per core, BN stats/apply +
  residual glue between conv launches, final densification.
- Device (Bass/Tile, SPMD on 8 NeuronCores): one generic 27-tap sparse-conv
  kernel — dma_gather (transposing, bf16) of neighbor rows + per-tap matmuls
  accumulated in PSUM (fp32) — reused for every conv in the network.
"""
import sys
import numpy as np

sys.path.insert(0, "/opt/trn_rl_repo")
sys.path.insert(0, "/root/.axon_site")

import ml_dtypes

BF16 = ml_dtypes.bfloat16

N_CORES = 8
GRID0 = (40, 1504, 1504)
OFF27 = [(dz, dy, dx) for dz in (-1, 0, 1) for dy in (-1, 0, 1) for dx in (-1, 0, 1)]
OFFZ = [(-1, 0, 0), (0, 0, 0), (1, 0, 0)]
TAPS = 27
M_PAD = 5120          # padded output rows per core (10 tiles x 512)
TILE = 512
NTILES = M_PAD // TILE
R_PAD = 8192          # padded input rows per core (row 0 = zeros)
IDXW = TILE * TAPS // 16   # wrapped idx columns per tile (13824/16 = 864)

_nc_cache = {}


# ---------------------------------------------------------------- host tables
def _flat(c, g):
    return ((c[:, 0].astype(np.int64) * g[0] + c[:, 1]) * g[1] + c[:, 2]) * g[2] + c[:, 3]


class _Lv:
    def __init__(self, coords, g, orig):
        self.coords = coords
        self.g = g
        self.orig = orig
        f = _flat(coords, g)
        order = np.argsort(f, kind="stable")
        self.sf = f[order]
        self.sr = order.astype(np.int32)

    def lookup(self, f):
        pos = np.searchsorted(self.sf, f)
        pos_c = np.minimum(pos, len(self.sf) - 1)
        hit = self.sf[pos_c] == f
        return np.where(hit, self.sr[pos_c], -1).astype(np.int32)


def _downsample(lv, stride):
    g_out = tuple(-(-lv.g[i] // stride[i]) for i in range(3))
    q = lv.coords // np.array([1, stride[0], stride[1], stride[2]], np.int32)
    f = _flat(q, g_out)
    order = np.lexsort((lv.orig, f))
    fs = f[order]
    last = np.ones(len(fs), bool)
    last[:-1] = fs[:-1] != fs[1:]
    canon = order[last]
    canon.sort()
    return _Lv(q[canon], g_out, lv.orig[canon])


def _conv_idx(out_lv, in_lv, offsets, stride):
    base = out_lv.coords * np.array([1, stride[0], stride[1], stride[2]], np.int32)
    g = in_lv.g
    idxs = []
    for off in offsets:
        nb = base + np.array([0, off[0], off[1], off[2]], np.int32)
        ok = ((nb[:, 1] >= 0) & (nb[:, 1] < g[0]) &
              (nb[:, 2] >= 0) & (nb[:, 2] < g[1]) &
              (nb[:, 3] >= 0) & (nb[:, 3] < g[2]))
        f = _flat(np.where(ok[:, None], nb, 0), g)
        idx = in_lv.lookup(f)
        idxs.append(np.where(ok, idx, -1).astype(np.int32))
    return np.stack(idxs)


# ------------------------------------------------------------- device program
def _get_nc():
    if "nc" in _nc_cache:
        return _nc_cache["nc"]
    from concourse import bacc, tile, mybir

    nc = bacc.Bacc("TRN2", target_bir_lowering=False, debug=False,
                   num_devices=N_CORES)
    xrow = nc.dram_tensor("xrow", [R_PAD, 128], mybir.dt.bfloat16,
                          kind="ExternalInput")
    wts = nc.dram_tensor("wts", [128, TAPS * 128], mybir.dt.bfloat16,
                         kind="ExternalInput")
    idx = nc.dram_tensor("idx", [128, NTILES * IDXW], mybir.dt.int16,
                         kind="ExternalInput")
    out = nc.dram_tensor("out", [128, M_PAD], mybir.dt.float32,
                         kind="ExternalOutput")

    with tile.TileContext(nc) as tc:
        with tc.tile_pool(name="const", bufs=1) as cpool, \
             tc.tile_pool(name="work", bufs=2) as wpool, \
             tc.tile_pool(name="ps", bufs=2, space="PSUM") as ppool:
            wsb = cpool.tile([128, TAPS * 128], mybir.dt.bfloat16)
            nc.sync.dma_start(wsb[:], wts.ap())
            isb = cpool.tile([128, NTILES * IDXW], mybir.dt.int16)
            nc.sync.dma_start(isb[:], idx.ap())
            for t in range(NTILES):
                g = wpool.tile([128, 1, TAPS * TILE], mybir.dt.bfloat16,
                               tag="gath")
                nc.gpsimd.dma_gather(
                    g[:, :, :], xrow.ap(),
                    isb[:, t * IDXW:(t + 1) * IDXW],
                    num_idxs=TAPS * TILE, num_idxs_reg=TAPS * TILE,
                    elem_size=128, transpose=True, single_packet=False)
                ps = ppool.tile([128, TILE], mybir.dt.float32)
                for k in range(TAPS):
                    nc.tensor.matmul(ps[:], wsb[:, k * 128:(k + 1) * 128],
                                     g[:, 0, k * TILE:(k + 1) * TILE],
                                     start=(k == 0), stop=(k == TAPS - 1))
                ob = wpool.tile([128, TILE], mybir.dt.float32, tag="ob")
                nc.any.tensor_copy(ob[:], ps[:])
                nc.sync.dma_start(out.ap()[:, t * TILE:(t + 1) * TILE], ob[:])
    nc.compile()
    _nc_cache["nc"] = nc
    return nc


def _wrap_idx(arr):
    """arr: (TAPS*TILE,) int -> wrapped [128, IDXW] int16 (16-partition wrap,
    replicated to all 8 Q7 core groups)."""
    w = arr.astype(np.int16).reshape(IDXW, 16).T
    return np.tile(w, (8, 1))


def _run_conv(x_bf16, idxs, W, order=None):
    """x_bf16: (N_in, Cin) bf16; idxs: (K, M) int32 global; W: (K, Cin, Cout) f32.
    order: spatial permutation of output rows used for sharding locality.
    Returns raw conv output (M, Cout) float32 computed on the 8 NeuronCores."""
    from concourse import bass_utils

    K, M = idxs.shape
    Cin, Cout = W.shape[1], W.shape[2]
    assert K <= TAPS and M <= N_CORES * M_PAD and Cin <= 128 and Cout <= 128
    if order is None:
        order = np.arange(M)

    wpad = np.zeros((TAPS, 128, 128), np.float32)
    wpad[:K, :Cin, :Cout] = W
    wpad = np.ascontiguousarray(wpad.transpose(1, 0, 2).reshape(128, TAPS * 128)
                                ).astype(BF16)

    bounds = [M * c // N_CORES for c in range(N_CORES + 1)]
    in_maps = []
    parts = []
    for c in range(N_CORES):
        rows = order[bounds[c]:bounds[c + 1]]
        sub = idxs[:, rows]                       # (K, m)
        need = np.unique(sub[sub >= 0])
        lut = np.zeros(x_bf16.shape[0] + 1, np.int32)
        lut[need] = np.arange(1, len(need) + 1, dtype=np.int32)
        loc = np.where(sub >= 0, lut[np.maximum(sub, 0)], 0)  # (K, m) local
        assert len(need) + 1 <= R_PAD, len(need)

        xr = np.zeros((R_PAD, 128), BF16)
        xr[1:1 + len(need), :Cin] = x_bf16[need]

        m = len(rows)
        full = np.zeros((TAPS, M_PAD), np.int32)
        full[:K, :m] = loc
        idx_in = np.zeros((NTILES, 128, IDXW), np.int16)
        for t in range(NTILES):
            concat = full[:, t * TILE:(t + 1) * TILE].reshape(TAPS * TILE)
            idx_in[t] = _wrap_idx(concat)
        idx_in = np.ascontiguousarray(
            idx_in.transpose(1, 0, 2).reshape(128, NTILES * IDXW))
        in_maps.append({"xrow": xr, "wts": wpad, "idx": idx_in})
        parts.append((rows, bounds[c + 1] - bounds[c]))

    nc = _get_nc()
    _nc_cache["last_maps"] = in_maps
    res = bass_utils.run_bass_kernel_spmd(nc, in_maps,
                                          core_ids=list(range(N_CORES)))
    outp = np.empty((M, Cout), np.float32)
    for c in range(N_CORES):
        rows, m = parts[c]
        outp[rows] = res.results[c]["out"][:Cout, :m].T
    return outp


# ---------------------------------------------------------------- forward net
def _bn_relu(raw, g, b, relu=True, res=None, eps=1e-5):
    n = np.float32(raw.shape[0])
    mu = raw.sum(0, dtype=np.float64).astype(np.float32) / n
    d = raw - mu
    var = (d * d).sum(0, dtype=np.float64).astype(np.float32) / n
    y = d * (1.0 / np.sqrt(var + eps)) * g + b
    if res is not None:
        y = y + res
    if relu:
        y = np.maximum(y, 0)
    return y


def kernel(features, coordinates, params, batch_size, input_shape):
    feats = np.asarray(features, np.float32)
    coords = np.asarray(coordinates, np.int32)
    P = params

    def A(x):
        return np.asarray(x, np.float32)

    lv0 = _Lv(coords, GRID0, np.arange(coords.shape[0], dtype=np.int64))
    lv1 = _downsample(lv0, (2, 2, 2))
    lv2 = _downsample(lv1, (2, 2, 2))
    lv3 = _downsample(lv2, (2, 2, 2))
    lv4 = _downsample(lv3, (3, 1, 1))

    i0 = _conv_idx(lv0, lv0, OFF27, (1, 1, 1))
    i01 = _conv_idx(lv1, lv0, OFF27, (2, 2, 2))
    i1 = _conv_idx(lv1, lv1, OFF27, (1, 1, 1))
    i12 = _conv_idx(lv2, lv1, OFF27, (2, 2, 2))
    i2 = _conv_idx(lv2, lv2, OFF27, (1, 1, 1))
    i23 = _conv_idx(lv3, lv2, OFF27, (2, 2, 2))
    i3 = _conv_idx(lv3, lv3, OFF27, (1, 1, 1))
    i34 = _conv_idx(lv4, lv3, OFFZ, (3, 1, 1))

    def yord(lv):
        return np.argsort(lv.coords[:, 2], kind="stable")

    o0, o1, o2, o3, o4 = (yord(lv) for lv in (lv0, lv1, lv2, lv3, lv4))

    def conv_bn(x, idxs, W, g, b, order, relu=True, res=None):
        raw = _run_conv(x.astype(BF16), idxs, A(W), order=order)
        return _bn_relu(raw, A(g), A(b), relu=relu, res=res)

    def block(x, idxs, bp, order):
        h = conv_bn(x, idxs, bp["W1"], bp["g1"], bp["b1"], order, relu=True)
        return conv_bn(h, idxs, bp["W2"], bp["g2"], bp["b2"], order,
                       relu=True, res=x)

    pi = P["conv_input"]
    x = conv_bn(feats, i0, pi["W"], pi["g"], pi["b"], o0)
    for bp in P["conv1"]:
        x = block(x, i0, bp, o0)
    for sidx, bidx, sp, so in [(i01, i1, P["conv2"], o1),
                               (i12, i2, P["conv3"], o2),
                               (i23, i3, P["conv4"], o3)]:
        x = conv_bn(x, sidx, sp["W"], sp["g"], sp["b"], so)
        for bp in sp["blocks"]:
            x = block(x, bidx, bp, so)
    pe = P["extra"]
    x = conv_bn(x, i34, pe["W"], pe["g"], pe["b"], o4)

    g4 = lv4.g
    c4 = lv4.coords
    dense = np.zeros((1, g4[0], g4[1], g4[2], x.shape[1]), np.float32)
    dense[c4[:, 0], c4[:, 1], c4[:, 2], c4[:, 3]] = x
    return np.transpose(dense, (0, 4, 1, 2, 3)).reshape(
        1, x.shape[1] * g4[0], g4[1], g4[2])


# revision 14
# speedup vs baseline: 2.3603x; 2.3603x over previous
"""Trainium2 Bass kernel for nn_MinkowskiSpMiddleResNetFHD (sparse 3D ResNet).

Strategy (v0):
- Host (numpy): build voxel hash tables / kernel maps per level, shard output
  rows across 8 cores, localize gather indices per core, BN stats/apply +
  residual glue between conv launches, final densification.
- Device (Bass/Tile, SPMD on 8 NeuronCores): one generic 27-tap sparse-conv
  kernel — dma_gather (transposing, bf16) of neighbor rows + per-tap matmuls
  accumulated in PSUM (fp32) — reused for every conv in the network.
"""
import sys
import numpy as np

sys.path.insert(0, "/opt/trn_rl_repo")
sys.path.insert(0, "/root/.axon_site")

import ml_dtypes

BF16 = ml_dtypes.bfloat16

N_CORES = 8
GRID0 = (40, 1504, 1504)
OFF27 = [(dz, dy, dx) for dz in (-1, 0, 1) for dy in (-1, 0, 1) for dx in (-1, 0, 1)]
OFFZ = [(-1, 0, 0), (0, 0, 0), (1, 0, 0)]
TAPS = 27
M_PAD = 5120          # padded output rows per core (10 tiles x 512)
TILE = 512
NTILES = M_PAD // TILE
R_PAD = 8192          # padded input rows per core (row 0 = zeros)
IDXW = TILE * TAPS // 16   # wrapped idx columns per tile (13824/16 = 864)

_nc_cache = {}


# ---------------------------------------------------------------- host tables
def _flat(c, g):
    return ((c[:, 0].astype(np.int64) * g[0] + c[:, 1]) * g[1] + c[:, 2]) * g[2] + c[:, 3]


class _Lv:
    def __init__(self, coords, g, orig):
        self.coords = coords
        self.g = g
        self.orig = orig
        f = _flat(coords, g)
        order = np.argsort(f, kind="stable")
        self.sf = f[order]
        self.sr = order.astype(np.int32)

    def lookup(self, f):
        pos = np.searchsorted(self.sf, f)
        pos_c = np.minimum(pos, len(self.sf) - 1)
        hit = self.sf[pos_c] == f
        return np.where(hit, self.sr[pos_c], -1).astype(np.int32)


def _downsample(lv, stride):
    g_out = tuple(-(-lv.g[i] // stride[i]) for i in range(3))
    q = lv.coords // np.array([1, stride[0], stride[1], stride[2]], np.int32)
    f = _flat(q, g_out)
    order = np.lexsort((lv.orig, f))
    fs = f[order]
    last = np.ones(len(fs), bool)
    last[:-1] = fs[:-1] != fs[1:]
    canon = order[last]
    canon.sort()
    return _Lv(q[canon], g_out, lv.orig[canon])


def _conv_idx(out_lv, in_lv, offsets, stride):
    base = out_lv.coords * np.array([1, stride[0], stride[1], stride[2]], np.int32)
    g = in_lv.g
    idxs = []
    for off in offsets:
        nb = base + np.array([0, off[0], off[1], off[2]], np.int32)
        ok = ((nb[:, 1] >= 0) & (nb[:, 1] < g[0]) &
              (nb[:, 2] >= 0) & (nb[:, 2] < g[1]) &
              (nb[:, 3] >= 0) & (nb[:, 3] < g[2]))
        f = _flat(np.where(ok[:, None], nb, 0), g)
        idx = in_lv.lookup(f)
        idxs.append(np.where(ok, idx, -1).astype(np.int32))
    return np.stack(idxs)


# ------------------------------------------------------------- device program
def _get_nc():
    if "nc" in _nc_cache:
        return _nc_cache["nc"]
    from concourse import bacc, tile, mybir

    nc = bacc.Bacc("TRN2", target_bir_lowering=False, debug=False,
                   num_devices=N_CORES)
    xrow = nc.dram_tensor("xrow", [R_PAD, 128], mybir.dt.bfloat16,
                          kind="ExternalInput")
    wts = nc.dram_tensor("wts", [128, TAPS * 128], mybir.dt.bfloat16,
                         kind="ExternalInput")
    idx = nc.dram_tensor("idx", [128, NTILES * IDXW], mybir.dt.int16,
                         kind="ExternalInput")
    out = nc.dram_tensor("out", [128, M_PAD], mybir.dt.float32,
                         kind="ExternalOutput")

    with tile.TileContext(nc) as tc:
        with tc.tile_pool(name="const", bufs=1) as cpool, \
             tc.tile_pool(name="work", bufs=2) as wpool, \
             tc.tile_pool(name="ps", bufs=2, space="PSUM") as ppool:
            wsb = cpool.tile([128, TAPS * 128], mybir.dt.bfloat16)
            nc.sync.dma_start(wsb[:], wts.ap())
            isb = cpool.tile([128, NTILES * IDXW], mybir.dt.int16)
            nc.sync.dma_start(isb[:], idx.ap())
            for t in range(NTILES):
                g = wpool.tile([128, 1, TAPS * TILE], mybir.dt.bfloat16,
                               tag="gath")
                nc.gpsimd.dma_gather(
                    g[:, :, :], xrow.ap(),
                    isb[:, t * IDXW:(t + 1) * IDXW],
                    num_idxs=TAPS * TILE, num_idxs_reg=TAPS * TILE,
                    elem_size=128, transpose=True, single_packet=False)
                ps = ppool.tile([128, TILE], mybir.dt.float32)
                for k in range(TAPS):
                    nc.tensor.matmul(ps[:], wsb[:, k * 128:(k + 1) * 128],
                                     g[:, 0, k * TILE:(k + 1) * TILE],
                                     start=(k == 0), stop=(k == TAPS - 1))
                ob = wpool.tile([128, TILE], mybir.dt.float32, tag="ob")
                nc.any.tensor_copy(ob[:], ps[:])
                nc.sync.dma_start(out.ap()[:, t * TILE:(t + 1) * TILE], ob[:])
    nc.compile()
    _nc_cache["nc"] = nc
    return nc


def _wrap_idx(arr):
    """arr: (TAPS*TILE,) int -> wrapped [128, IDXW] int16 (16-partition wrap,
    replicated to all 8 Q7 core groups)."""
    w = arr.astype(np.int16).reshape(IDXW, 16).T
    return np.tile(w, (8, 1))


def _run_conv(x_bf16, idxs, W, order=None):
    """x_bf16: (N_in, Cin) bf16; idxs: (K, M) int32 global; W: (K, Cin, Cout) f32.
    order: spatial permutation of output rows used for sharding locality.
    Returns raw conv output (M, Cout) float32 computed on the 8 NeuronCores."""
    from concourse import bass_utils

    K, M = idxs.shape
    Cin, Cout = W.shape[1], W.shape[2]
    assert K <= TAPS and M <= N_CORES * M_PAD and Cin <= 128 and Cout <= 128
    if order is None:
        order = np.arange(M)

    wpad = np.zeros((TAPS, 128, 128), np.float32)
    wpad[:K, :Cin, :Cout] = W
    wpad = np.ascontiguousarray(wpad.transpose(1, 0, 2).reshape(128, TAPS * 128)
                                ).astype(BF16)

    bounds = [M * c // N_CORES for c in range(N_CORES + 1)]
    in_maps = []
    parts = []
    for c in range(N_CORES):
        rows = order[bounds[c]:bounds[c + 1]]
        sub = idxs[:, rows]                       # (K, m)
        need = np.unique(sub[sub >= 0])
        # rows 0..127 are all-zero sentinel targets: spreading sentinel
        # gathers over 128 distinct rows avoids HBM bank congestion from
        # hammering a single row.
        NZ = 128
        lut = np.zeros(x_bf16.shape[0] + 1, np.int32)
        lut[need] = np.arange(NZ, NZ + len(need), dtype=np.int32)
        assert len(need) + NZ <= R_PAD, len(need)

        xr = np.zeros((R_PAD, 128), BF16)
        xr[NZ:NZ + len(need), :Cin] = x_bf16[need]

        m = len(rows)
        spread = np.broadcast_to(
            np.arange(M_PAD, dtype=np.int32) % NZ, (TAPS, M_PAD))
        full = spread.copy()
        loc = np.where(sub >= 0, lut[np.maximum(sub, 0)], spread[:K, :m])
        full[:K, :m] = loc
        idx_in = np.zeros((NTILES, 128, IDXW), np.int16)
        for t in range(NTILES):
            concat = full[:, t * TILE:(t + 1) * TILE].reshape(TAPS * TILE)
            idx_in[t] = _wrap_idx(concat)
        idx_in = np.ascontiguousarray(
            idx_in.transpose(1, 0, 2).reshape(128, NTILES * IDXW))
        in_maps.append({"xrow": xr, "wts": wpad, "idx": idx_in})
        parts.append((rows, bounds[c + 1] - bounds[c]))

    nc = _get_nc()
    _nc_cache["last_maps"] = in_maps
    res = bass_utils.run_bass_kernel_spmd(nc, in_maps,
                                          core_ids=list(range(N_CORES)))
    outp = np.empty((M, Cout), np.float32)
    for c in range(N_CORES):
        rows, m = parts[c]
        outp[rows] = res.results[c]["out"][:Cout, :m].T
    return outp


# ---------------------------------------------------------------- forward net
def _bn_relu(raw, g, b, relu=True, res=None, eps=1e-5):
    n = np.float32(raw.shape[0])
    mu = raw.sum(0, dtype=np.float64).astype(np.float32) / n
    d = raw - mu
    var = (d * d).sum(0, dtype=np.float64).astype(np.float32) / n
    y = d * (1.0 / np.sqrt(var + eps)) * g + b
    if res is not None:
        y = y + res
    if relu:
        y = np.maximum(y, 0)
    return y


def kernel(features, coordinates, params, batch_size, input_shape):
    feats = np.asarray(features, np.float32)
    coords = np.asarray(coordinates, np.int32)
    P = params

    def A(x):
        return np.asarray(x, np.float32)

    lv0 = _Lv(coords, GRID0, np.arange(coords.shape[0], dtype=np.int64))
    lv1 = _downsample(lv0, (2, 2, 2))
    lv2 = _downsample(lv1, (2, 2, 2))
    lv3 = _downsample(lv2, (2, 2, 2))
    lv4 = _downsample(lv3, (3, 1, 1))

    i0 = _conv_idx(lv0, lv0, OFF27, (1, 1, 1))
    i01 = _conv_idx(lv1, lv0, OFF27, (2, 2, 2))
    i1 = _conv_idx(lv1, lv1, OFF27, (1, 1, 1))
    i12 = _conv_idx(lv2, lv1, OFF27, (2, 2, 2))
    i2 = _conv_idx(lv2, lv2, OFF27, (1, 1, 1))
    i23 = _conv_idx(lv3, lv2, OFF27, (2, 2, 2))
    i3 = _conv_idx(lv3, lv3, OFF27, (1, 1, 1))
    i34 = _conv_idx(lv4, lv3, OFFZ, (3, 1, 1))

    def yord(lv):
        return np.argsort(lv.coords[:, 2], kind="stable")

    o0, o1, o2, o3, o4 = (yord(lv) for lv in (lv0, lv1, lv2, lv3, lv4))

    def conv_bn(x, idxs, W, g, b, order, relu=True, res=None):
        raw = _run_conv(x.astype(BF16), idxs, A(W), order=order)
        return _bn_relu(raw, A(g), A(b), relu=relu, res=res)

    def block(x, idxs, bp, order):
        h = conv_bn(x, idxs, bp["W1"], bp["g1"], bp["b1"], order, relu=True)
        return conv_bn(h, idxs, bp["W2"], bp["g2"], bp["b2"], order,
                       relu=True, res=x)

    pi = P["conv_input"]
    x = conv_bn(feats, i0, pi["W"], pi["g"], pi["b"], o0)
    for bp in P["conv1"]:
        x = block(x, i0, bp, o0)
    for sidx, bidx, sp, so in [(i01, i1, P["conv2"], o1),
                               (i12, i2, P["conv3"], o2),
                               (i23, i3, P["conv4"], o3)]:
        x = conv_bn(x, sidx, sp["W"], sp["g"], sp["b"], so)
        for bp in sp["blocks"]:
            x = block(x, bidx, bp, so)
    pe = P["extra"]
    x = conv_bn(x, i34, pe["W"], pe["g"], pe["b"], o4)

    g4 = lv4.g
    c4 = lv4.coords
    dense = np.zeros((1, g4[0], g4[1], g4[2], x.shape[1]), np.float32)
    dense[c4[:, 0], c4[:, 1], c4[:, 2], c4[:, 3]] = x
    return np.transpose(dense, (0, 4, 1, 2, 3)).reshape(
        1, x.shape[1] * g4[0], g4[1], g4[2])
